# Initial kernel scaffold
#
"""Masked-fill kernel for Trainium2 (8 NeuronCores, data-parallel over batch).

reference semantics:
    masked_x = x.copy(); masked_x[b, mask_indices[b], :] = emb_mask[0]
    return masked_x, mask_indices

Strategy: shard x along batch (4 rows per core). On the host, expand
mask_indices into a dense {0,1} uint32 mask over the N axis (cheap: 256K
entries). On-device, each core streams its 32 MiB shard through SBUF in
4 MiB tiles and applies a single DVE copy_predicated per tile (mask
broadcast over DIM, emb broadcast over the row-chunk axis) between the
load and the store. Total HBM traffic per core = 32 MiB read + 32 MiB
write + ~0.25 MiB of mask/emb: the memory roofline for this op.
"""

import numpy as np

B, N, DIM = 32, 8192, 256
N_CORES = 8
RPC = B // N_CORES  # batch rows per core = 4
P = 128             # SBUF partitions
CHUNKS = 2          # tiles per batch row
NH = N // CHUNKS    # 4096 rows of x per tile
T = NH // P         # 32 rows per partition per tile
MASK_COLS = RPC * CHUNKS * T  # 256

_cached = {}


def _build_nc():
    import concourse.bacc as bacc
    import concourse.mybir as mybir
    import concourse.tile as tile

    nc = bacc.Bacc("TRN2", target_bir_lowering=False, debug=False)
    f32 = mybir.dt.float32
    u32 = mybir.dt.uint32

    x_in = nc.dram_tensor("x", [RPC, N, DIM], f32, kind="ExternalInput")
    mask_in = nc.dram_tensor("mask", [P, MASK_COLS], u32, kind="ExternalInput")
    emb_in = nc.dram_tensor("emb", [P, DIM], f32, kind="ExternalInput")
    out = nc.dram_tensor("out", [RPC, N, DIM], f32, kind="ExternalOutput")

    with tile.TileContext(nc) as tc:
        with (
            tc.tile_pool(name="const", bufs=1) as cpool,
            tc.tile_pool(name="big", bufs=5) as pool,
        ):
            mask_t = cpool.tile([P, MASK_COLS], u32)
            emb_t = cpool.tile([P, DIM], f32)
            nc.sync.dma_start(out=mask_t[:], in_=mask_in[:])
            nc.sync.dma_start(out=emb_t[:], in_=emb_in[:])

            for r in range(RPC):
                for h in range(CHUNKS):
                    buf = pool.tile([P, T * DIM], f32, tag="xbuf")
                    dst3 = buf[:].rearrange("p (t d) -> p t d", d=DIM)
                    src = x_in[r, h * NH:(h + 1) * NH, :].rearrange(
                        "(p t) d -> p t d", p=P
                    )
                    nc.sync.dma_start(out=dst3, in_=src)

                    c0 = (r * CHUNKS + h) * T
                    nc.vector.copy_predicated(
                        out=dst3,
                        mask=mask_t[:, c0:c0 + T].unsqueeze(2).broadcast_to(
                            [P, T, DIM]
                        ),
                        data=emb_t[:].unsqueeze(1).broadcast_to([P, T, DIM]),
                    )

                    dsto = out[r, h * NH:(h + 1) * NH, :].rearrange(
                        "(p t) d -> p t d", p=P
                    )
                    nc.scalar.dma_start(out=dsto, in_=dst3)
    return nc


def _get_nc():
    if "nc" not in _cached:
        _cached["nc"] = _build_nc()
    return _cached["nc"]


def _host_prep(x, mask_indices, emb_mask):
    """Build per-core input maps."""
    dense = np.zeros((B, N), dtype=np.uint32)
    dense[np.arange(B)[:, None], mask_indices.astype(np.int64)] = 1
    emb_b = np.ascontiguousarray(
        np.broadcast_to(emb_mask.astype(np.float32), (P, DIM))
    )
    in_maps = []
    for c in range(N_CORES):
        xs = np.ascontiguousarray(x[c * RPC:(c + 1) * RPC], dtype=np.float32)
        # device mask layout: [P, (r, h, t)] with row n = h*NH + p*T + t
        ms = dense[c * RPC:(c + 1) * RPC].reshape(RPC, CHUNKS, P, T)
        ms = np.ascontiguousarray(ms.transpose(2, 0, 1, 3).reshape(P, MASK_COLS))
        in_maps.append({"x": xs, "mask": ms, "emb": emb_b})
    return in_maps


def kernel(x, mask_indices, emb_mask):
    from concourse.bass_utils import run_bass_kernel_spmd

    nc = _get_nc()
    in_maps = _host_prep(
        np.asarray(x), np.asarray(mask_indices), np.asarray(emb_mask)
    )
    res = run_bass_kernel_spmd(nc, in_maps, core_ids=list(range(N_CORES)))
    masked = np.concatenate([r["out"] for r in res.results], axis=0)
    return masked, np.asarray(mask_indices)


# revision 4
# speedup vs baseline: 1.1727x; 1.1727x over previous
"""Masked-fill kernel for Trainium2 (8 NeuronCores, data-parallel over batch).

reference semantics:
    masked_x = x.copy(); masked_x[b, mask_indices[b], :] = emb_mask[0]
    return masked_x, mask_indices

Strategy: shard x along batch (4 rows per core). On the host, expand
mask_indices into a dense {0,1} uint32 mask over the N axis (cheap: 256K
entries). On-device, each core streams its 32 MiB shard through SBUF in
4 MiB tiles and applies a single DVE copy_predicated per tile (mask
broadcast over DIM, emb broadcast over the row-chunk axis) between the
load and the store. Total HBM traffic per core = 32 MiB read + 32 MiB
write + ~0.25 MiB of mask/emb: the memory roofline for this op.
"""

import numpy as np

B, N, DIM = 32, 8192, 256
N_CORES = 8
RPC = B // N_CORES  # batch rows per core = 4
P = 128             # SBUF partitions
CHUNKS = 2          # tiles per batch row
NH = N // CHUNKS    # 4096 rows of x per tile
T = NH // P         # 32 rows per partition per tile
MASK_COLS = RPC * CHUNKS * T  # 256

_cached = {}


def _build_nc(n_reps=1):
    import concourse.bacc as bacc
    import concourse.mybir as mybir
    import concourse.tile as tile

    nc = bacc.Bacc("TRN2", target_bir_lowering=False, debug=False)
    f32 = mybir.dt.float32
    u32 = mybir.dt.uint32

    x_in = nc.dram_tensor("x", [RPC, N, DIM], f32, kind="ExternalInput")
    mask_in = nc.dram_tensor("mask", [P, MASK_COLS], u32, kind="ExternalInput")
    emb_in = nc.dram_tensor("emb", [P, DIM], f32, kind="ExternalInput")
    out = nc.dram_tensor("out", [RPC, N, DIM], f32, kind="ExternalOutput")
    # internal scratch outputs for benchmark repetitions (n_reps > 1)
    scratch = [
        nc.dram_tensor(f"scratch{i}", [RPC, N, DIM], f32) for i in range(2)
    ] if n_reps > 1 else []

    with tile.TileContext(nc) as tc:
        with (
            tc.tile_pool(name="const", bufs=1) as cpool,
            tc.tile_pool(name="big", bufs=5) as pool,
        ):
            mask_t = cpool.tile([P, MASK_COLS], u32)
            emb_t = cpool.tile([P, DIM], f32)
            nc.sync.dma_start(out=mask_t[:], in_=mask_in[:])
            nc.sync.dma_start(out=emb_t[:], in_=emb_in[:])

            for rep in range(n_reps):
                tgt = out if rep == n_reps - 1 else scratch[rep % 2]
                src_t = x_in if rep == 0 else scratch[(rep - 1) % 2]
                for r in range(RPC):
                    for h in range(CHUNKS):
                        buf = pool.tile([P, T * DIM], f32, tag="xbuf")
                        dst3 = buf[:].rearrange("p (t d) -> p t d", d=DIM)
                        src = src_t[r, h * NH:(h + 1) * NH, :].rearrange(
                            "(p t) d -> p t d", p=P
                        )
                        nc.sync.dma_start(out=dst3, in_=src)

                        c0 = (r * CHUNKS + h) * T
                        nc.vector.copy_predicated(
                            out=dst3,
                            mask=mask_t[:, c0:c0 + T].unsqueeze(2).broadcast_to(
                                [P, T, DIM]
                            ),
                            data=emb_t[:].unsqueeze(1).broadcast_to([P, T, DIM]),
                        )

                        dsto = tgt[r, h * NH:(h + 1) * NH, :].rearrange(
                            "(p t) d -> p t d", p=P
                        )
                        nc.scalar.dma_start(out=dsto, in_=dst3)
    nc.compile()
    return nc


def _get_nc():
    if "nc" not in _cached:
        _cached["nc"] = _build_nc()
    return _cached["nc"]


def _host_prep(x, mask_indices, emb_mask):
    """Build per-core input maps."""
    dense = np.zeros((B, N), dtype=np.uint32)
    dense[np.arange(B)[:, None], mask_indices.astype(np.int64)] = 1
    emb_b = np.ascontiguousarray(
        np.broadcast_to(emb_mask.astype(np.float32), (P, DIM))
    )
    in_maps = []
    for c in range(N_CORES):
        xs = np.ascontiguousarray(x[c * RPC:(c + 1) * RPC], dtype=np.float32)
        # device mask layout: [P, (r, h, t)] with row n = h*NH + p*T + t
        ms = dense[c * RPC:(c + 1) * RPC].reshape(RPC, CHUNKS, P, T)
        ms = np.ascontiguousarray(ms.transpose(2, 0, 1, 3).reshape(P, MASK_COLS))
        in_maps.append({"x": xs, "mask": ms, "emb": emb_b})
    return in_maps


def kernel(x, mask_indices, emb_mask):
    from concourse.bass_utils import run_bass_kernel_spmd

    nc = _get_nc()
    in_maps = _host_prep(
        np.asarray(x), np.asarray(mask_indices), np.asarray(emb_mask)
    )
    res = run_bass_kernel_spmd(nc, in_maps, core_ids=list(range(N_CORES)))
    masked = np.concatenate([r["out"] for r in res.results], axis=0)
    return masked, np.asarray(mask_indices)
